# revision 32
# baseline (speedup 1.0000x reference)
"""Trainium2 Bass kernel for nn_MixtureAttention.

Math: the reference builds a (c,c) pairwise Cauchy-product matrix per batch,
row-normalizes it, and keeps only the diagonal.  With
    qn(i,j) = prod_d (1 + (mu[j,d]-mu[i,d])^2 / sig[i,d]^2)
the kept diagonal reduces to   coef[i] = 1 / sum_j 1/qn(i,j)
(`pi` cancels in the row normalization), and y[b,ch,c] = x[b,ch] * coef[b,c].

Kernel: qn = q01 * q23 where each pair-of-dims factor is a degree-(2,2)
polynomial in the point coordinates -> a K=9 feature matmul per pair.
Each fp32 feature is split into three fp16 parts (hi/mid/lo); the six
product combinations hh,hm,mh,hl,mm,lh are stacked along the contraction
dim (K=54) so ONE fp16 matmul per pair computes the full product to
~5e-10 relative (dropped terms ~2^-33).  The two pair matmuls run
row-tiled (contraction bases 0 and 64) so they overlap in the PE array.
Per (128-row, 1024-point) tile: ACT computes u01 = 1/q01 (raw Reciprocal,
~1.2e-5) PSUM->SBUF, DVE computes recip1NR(q23)*u01 with accumulated
row-sum in one fused custom op (~0.17% max, equioscillating).  The
epilogue runs per row block as soon as its four groups finish: coef =
1/S (ACT raw Reciprocal) and y emitted TRANSPOSED, yT[c, ch] =
coef[c] * x[ch], as per-partition-scalar DVE multiplies against a
host-broadcast x tile, written out as fp16 with contiguous DMAs (the
host casts and transposes during reassembly) -- no on-device transpose
or DRAM bounce anywhere, and the y traffic spreads across the whole
main loop.

Sharding: 8 cores; core k handles batch k//2, c-rows [(k%2)*2048, +2048).
"""

import numpy as np

B, C, D, CH = 4, 4096, 4, 256
NCORES = 8
CW = C // 2            # 2048 c-rows per core (2 cores per batch)
NBLK = CW // 128       # 16 row blocks
GW = 1024              # point-group width (2 PSUM banks per pair factor)
NG = C // GW           # 4 groups per row block
KS = 54                # stacked contraction dim (6 fp16-split combos x K=9)

_cache = {}


def _register_op(name, spec):
    """Register a custom DVE op into concourse's op table at runtime; uop
    shas are self-pinned by compiling once and reading the reported digest."""
    import re

    from concourse import dve_ops as DO

    key = "op_" + name
    if key in _cache:
        return _cache[key]
    shas = {}
    for ver in ("v3", "v4"):
        probe = DO.DveOp(name + "_PROBE", spec, subdim=False, uops_sha={})
        if name + "_PROBE" not in DO._SUB_OPCODE_FOR_NAME:
            DO._SUB_OPCODE_FOR_NAME[name + "_PROBE"] = 0x1F
        try:
            probe.compile(ver)
        except ValueError as e:
            m = re.search(r'"(?:v3|v4)"\]="([0-9a-f]+)"', str(e))
            if not m:
                raise
            shas[ver] = m.group(1)
    op = DO.DveOp(name, spec, subdim=False, uops_sha=shas)
    if name not in DO._SUB_OPCODE_FOR_NAME:
        DO.OPS.append(op)
        DO._SUB_OPCODE_FOR_NAME[name] = DO._CUSTOM_DVE_ROW_BASE + len(DO.OPS) - 1
        assert DO._SUB_OPCODE_FOR_NAME[name] < 0x20, "opcode rows exhausted"
    DO.CUSTOM_DVE_SPECS[name] = spec
    _cache[key] = op
    return op


def _np_nr1(x, c0, c1):
    nx = (~x.view(np.int32)).view(np.float32)
    y0 = (nx * np.float32(c0)).astype(np.float32)
    return (y0 * (np.float32(c1) - x * y0)).astype(np.float32)


# Chebyshev pair for the 1-NR fast reciprocal (same interval as concourse's
# RECIPROCAL_APPROX_FAST; one NR step -> ~1.7e-3 max, sign-balanced).
RC0, RC1 = -0.23549792, 2.0017324


def _get_rmacc():
    """out = recip1NR(Src0) * Src1, accum_out = row-sum(out).  7 DVE stages."""
    import operator

    from concourse.dve_spec import C0, C1, Bin, Spec, Src0, Src1, Zero
    from concourse.dve_uop import AluOp

    nx = Bin(AluOp.BITWISE_NOT, Src0, Src0)
    y0 = nx * C0
    y1 = y0 * (C1 - Src0 * y0)

    def _ref(in0, in1, c0, c1, c2):
        b = (_np_nr1(in0, c0, c1) * in1).astype(np.float32)
        return b, b.reshape(b.shape[0], -1).sum(axis=-1, keepdims=True)

    return _register_op(
        "RECIP1_MUL_ACC_ANT",
        Spec(body=y1 * Src1, accum=operator.add, accum_init=Zero, reference=_ref),
    )


def _build(bench_nrep=None, bench_span="full"):
    import concourse.bacc as bacc
    import concourse.mybir as mybir
    from concourse.tile import TileContext

    f32 = mybir.dt.float32
    f16 = mybir.dt.float16
    Act = mybir.ActivationFunctionType

    rmacc = _get_rmacc()
    nc = bacc.Bacc(None, target_bir_lowering=False)
    af = nc.declare_dram_parameter("af", [118, CW], f16, isOutput=False)
    pf = nc.declare_dram_parameter("pf", [118, C], f16, isOutput=False)
    xv = nc.declare_dram_parameter("xv", [128, CH], f32, isOutput=False)
    y = nc.declare_dram_parameter("y", [CW, CH], f16, isOutput=True)

    imm = lambda v: mybir.ImmediateValue(dtype=f32, value=v)

    with TileContext(nc) as tc:
        with (
            tc.tile_pool(name="persist", bufs=1) as pp,
            tc.tile_pool(name="work", bufs=1) as wp,
            tc.tile_pool(name="psum", bufs=2, space="PSUM") as psp,
            tc.tile_pool(name="dram", bufs=1, space="DRAM") as dp,
        ):
            af_s = pp.tile([118, 256], f16, name="afs")
            af_r = pp.tile([118, CW - 256], f16, name="afr")
            pf_s = pp.tile([118, GW], f16, name="pfs")
            pf_r = pp.tile([118, C - GW], f16, name="pfr")
            xbc = pp.tile([128, CH], f32)

            def _af(n, pb):
                if n < 2:
                    return af_s[:, n * 128:(n + 1) * 128]
                return af_r[:, (n - 2) * 128:(n - 1) * 128]

            def _pf(g, pb):
                if g == 0:
                    return pf_s
                return pf_r[:, (g - 1) * GW:g * GW]

            def load_inputs(pb):
                nc.scalar.dma_start(out=af_s[:, :], in_=af[:, 0:256])
                nc.sync.dma_start(out=pf_s[:, :], in_=pf[:, 0:GW])
                nc.scalar.dma_start(out=af_r[:, :], in_=af[:, 256:CW])
                nc.sync.dma_start(out=pf_r[:, 0:1536], in_=pf[:, GW:2560])
                nc.sync.dma_start(out=pf_r[:, 1536:3072],
                                  in_=pf[:, 2560:4096])
                nc.scalar.dma_start(out=xbc[:, :], in_=xv[:, :])

            Racc = pp.tile([128, NBLK, NG], f32)

            u01_const = pp.tile([128, 2, 512], f32, name="u01c")

            def main_loop(n_lo, n_hi, mode="all", pb=0):
                for n in range(n_lo, n_hi):
                    aft = _af(n, pb)
                    for g in range(NG):
                        p01 = psp.tile([128, 2, 512], f32, tag="p01", name="p01")
                        p23 = psp.tile([128, 2, 512], f32, tag="p23", name="p23")
                        for q in range(2):
                            sl = slice(q * 512, (q + 1) * 512)
                            nc.tensor.matmul(
                                p23[:, q, :], aft[64:64 + KS, :],
                                _pf(g, pb)[64:64 + KS, sl],
                                start=True, stop=True,
                            )
                            nc.tensor.matmul(
                                p01[:, q, :], aft[0:KS, :],
                                _pf(g, pb)[0:KS, sl],
                                start=True, stop=True,
                            )
                        eng = nc.scalar
                        if mode in ("all", "mmact"):
                            u01 = wp.tile([128, 2, 512], f32, tag="u01", bufs=4,
                                          name="u01")
                            eng.add_instruction(
                                mybir.InstActivation(
                                    name=nc.get_next_instruction_name(),
                                    func=Act.Reciprocal,
                                    ins=[eng.lower_ap(
                                        p01[:, :, :].rearrange("p a b -> p (a b)")),
                                         imm(0.0), imm(1.0), imm(0.0)],
                                    outs=[eng.lower_ap(
                                        u01[:, :, :].rearrange("p a b -> p (a b)"))],
                                )
                            )
                        if mode in ("all", "mmdve"):
                            src1 = u01 if mode == "all" else u01_const
                            junk = wp.tile([128, 2, 512], f32, tag="junk", bufs=3,
                                           name="junk")
                            nc.vector._custom_dve(
                                rmacc,
                                out=junk[:, :, :].rearrange("p a b -> p (a b)"),
                                in0=p23[:, :, :].rearrange("p a b -> p (a b)"),
                                in1=src1[:, :, :].rearrange("p a b -> p (a b)"),
                                s0=RC0, s1=RC1,
                                accum_out=Racc[:, n, g:g + 1],
                            )

            HB = NBLK // 2

            def rb_epilogue(n):
                Rsum = wp.tile([128, 1], f32, name="Rsum", tag="Rsum", bufs=2)
                nc.vector.tensor_reduce(
                    out=Rsum[:, :], in_=Racc[:, n, :],
                    axis=mybir.AxisListType.X, op=mybir.AluOpType.add,
                )
                coef = wp.tile([128, 1], f32, name="coef", tag="coef", bufs=2)
                eng = nc.scalar
                eng.add_instruction(
                    mybir.InstActivation(
                        name=nc.get_next_instruction_name(),
                        func=Act.Reciprocal,
                        ins=[eng.lower_ap(Rsum[:, :]), imm(0.0), imm(1.0),
                             imm(0.0)],
                        outs=[eng.lower_ap(coef[:, :])],
                    )
                )
                yt = wp.tile([128, CH], f16, tag="yt", bufs=4, name="yt")
                nc.scalar.activation(
                    yt[:, :], xbc[:, :], Act.Copy, scale=coef[:, 0:1],
                )
                nc.sync.dma_start(
                    out=y[n * 128:(n + 1) * 128, :], in_=yt[:, :],
                )

            def epilogue(half):
                for n in range(half * HB, (half + 1) * HB):
                    rb_epilogue(n)

            def whole(pb=0):
                load_inputs(pb)
                for n in range(NBLK):
                    main_loop(n, n + 1, pb=pb)
                    rb_epilogue(n)

            if bench_nrep is None:
                whole()
            elif bench_span == "full":
                U = 8
                if bench_nrep >= U:
                    with tc.For_i(0, bench_nrep // U, 1):
                        for u in range(U):
                            whole(pb=u % 2)
                else:
                    with tc.For_i(0, bench_nrep, 1):
                        whole()
            elif bench_span == "fullsr":
                import concourse.mybir as _mb

                with tc.For_i(
                    0, bench_nrep, 1,
                    staggered_reset=True,
                    hint_engines=(_mb.EngineType.DVE, _mb.EngineType.Activation),
                ):
                    whole()
            elif bench_span in ("mm", "mmact", "mmdve", "main"):
                load_inputs()
                nc.sync.dma_start(
                    out=u01_const[:, :, :].rearrange("p a b -> p (a b)")
                    .bitcast(f16),
                    in_=y[0:1024, :].rearrange("(p a) ch -> p (a ch)", p=128),
                )
                with tc.For_i(0, bench_nrep, 1):
                    main_loop(0, NBLK, mode="all" if bench_span == "main"
                              else ("mm" if bench_span == "mm" else bench_span))
                if bench_span in ("main", "mmdve"):
                    epilogue(0)
                    epilogue(1)
            elif bench_span == "load":
                with tc.For_i(0, bench_nrep, 1):
                    load_inputs()
                main_loop(0, NBLK)
                epilogue(0)
                epilogue(1)
            elif bench_span == "epi":
                load_inputs()
                main_loop(0, NBLK)
                with tc.For_i(0, bench_nrep, 1):
                    epilogue(0)
                    epilogue(1)
    nc.finalize()
    return nc


def _get_nc():
    if "nc" not in _cache:
        _cache["nc"] = _build()
    return _cache["nc"]


def _split3_f16(X):
    h = X.astype(np.float16)
    m = (X - h.astype(np.float64)).astype(np.float16)
    l = (X - h.astype(np.float64) - m.astype(np.float64)).astype(np.float16)
    return h, m, l


def _pair_features(mu_b, sig_b, rows, dims):
    """Stacked-K54 fp16 features: A [54, len(rows)], P [54, C]."""
    import itertools

    m = mu_b.astype(np.float64) - 0.5
    s2 = sig_b.astype(np.float64) ** 2
    cs = np.stack([(m * m + s2) / s2, -2 * m / s2, 1.0 / s2], axis=2)  # (C,D,3)
    fs = np.stack([np.ones_like(m), m, m * m], axis=2)                 # (C,D,3)
    d0, d1 = dims
    A = np.empty((9, len(rows)))
    P = np.empty((9, C))
    for k, (e0, e1) in enumerate(itertools.product(range(3), repeat=2)):
        A[k] = cs[rows, d0, e0] * cs[rows, d1, e1]
        P[k] = fs[:, d0, e0] * fs[:, d1, e1]
    # per-feature scale balancing keeps both sides in fp16 range and the
    # lo parts clear of subnormals
    s = np.sqrt(np.abs(P).max(axis=1) / np.abs(A).max(axis=1))
    A *= s[:, None]
    P /= s[:, None]
    Ah, Am, Al = _split3_f16(A)
    Ph, Pm, Pl = _split3_f16(P)
    # kept fp16-split products: hh, hm, mh, hl, mm, lh
    As = np.concatenate([Ah, Ah, Am, Ah, Am, Al], axis=0)
    Ps = np.concatenate([Ph, Pm, Ph, Pl, Pm, Ph], axis=0)
    return As, Ps


def _in_maps(x, mu, sig):
    maps = []
    for k in range(NCORES):
        b = k // 2
        half = k % 2
        rows = np.arange(half * CW, (half + 1) * CW)
        A01, P01 = _pair_features(mu[b], sig[b], rows, (0, 1))
        A23, P23 = _pair_features(mu[b], sig[b], rows, (2, 3))
        af = np.zeros((118, CW), np.float16)
        af[0:KS] = A01
        af[64:64 + KS] = A23
        pf = np.zeros((118, C), np.float16)
        pf[0:KS] = P01
        pf[64:64 + KS] = P23
        maps.append(
            {
                "af": af,
                "pf": pf,
                "xv": np.ascontiguousarray(
                    np.tile(np.asarray(x[b, :, 0], dtype=np.float32)[None, :],
                            (128, 1))
                ),
            }
        )
    return maps


def kernel(x, pi, mu, sig):
    from concourse.bass_utils import run_bass_kernel_spmd

    nc = _get_nc()
    res = run_bass_kernel_spmd(nc, _in_maps(x, mu, sig), list(range(NCORES))).results
    y = np.empty((B, CH, C), np.float32)
    for k in range(NCORES):
        b = k // 2
        half = k % 2
        y[b, :, half * CW:(half + 1) * CW] = res[k]["y"].astype(np.float32).T
    return y
